# revision 20
# baseline (speedup 1.0000x reference)
"""RBF-kernel causal attention on 8 Trainium2 NeuronCores.

B=2, H=16, N=2048, D=64. Shards the 32 (b,h) attention instances across 8
cores (4 heads per core). Math notes:

  logits = -relu(||q-k||^2)/sqrt(D); relu is a no-op (||q-k||^2 >= 0 up to
  rounding), and softmax is invariant to per-query offsets, so
      softmax_n(-(qsq_m + ksq_n - 2 qk)/8) == softmax_n(qk/4 - ksq_n/8)
  We compute P'' = exp(0.25 * K Q^T) in a [key, query] layout and fold the
  exp(-0.125 ksq_n) per-key factor into V (and into the appended ones-column
  that produces the softmax denominator):
      [O^T | l] accumulates via matmul(lhsT=V_aug_scaled, rhs=P'').
  Final output O[m,d] = OT[d,m] / l[m], un-transposed via PE transpose.

Emission is manually software-pipelined: head h+1's setup chunks (transposes,
ksq, V scaling) are interleaved between head h's query blocks so the tile
scheduler (limited lookahead) can overlap them.
"""

import sys

if "/opt/trn_rl_repo" not in sys.path:
    sys.path.insert(0, "/opt/trn_rl_repo")

import numpy as np

import concourse.bacc as bacc
import concourse.mybir as mybir
import concourse.tile as tile
from concourse.masks import make_identity

B, H, N, D = 2, 16, 2048, 64
NCORES = 8
HPC = (B * H) // NCORES  # heads per core = 4
P = 128                  # partitions
NT = N // P              # key tiles per head = 16
QB = 512                 # query block (matmul moving dim)
MBS = N // QB            # query blocks per head = 4
G = 2                    # key tiles per exp/ACT group (2 PSUM banks)

F32 = mybir.dt.float32
# float32r = relaxed-precision fp32 matmul (1 cycle/row at moving dim >= 256
# instead of 4 for float32)
MM_DT = mybir.dt.float32r


def build_nc():
    nc = bacc.Bacc("TRN2", target_bir_lowering=False, debug=False)
    q = nc.dram_tensor("q", [HPC, N, D], F32, kind="ExternalInput")
    k = nc.dram_tensor("k", [HPC, N, D], F32, kind="ExternalInput")
    v = nc.dram_tensor("v", [HPC, N, D], F32, kind="ExternalInput")
    out = nc.dram_tensor("out", [HPC, N, D], F32, kind="ExternalOutput")

    with tile.TileContext(nc) as tc:
        with (
            tc.tile_pool(name="const", bufs=1) as const_pool,
            tc.tile_pool(name="loads", bufs=1) as load_pool,
            tc.tile_pool(name="head", bufs=2) as head_pool,
            tc.tile_pool(name="work", bufs=3) as work_pool,
            tc.tile_pool(name="p", bufs=4) as p_pool,
            tc.tile_pool(name="epi", bufs=3) as epi_pool,
            tc.tile_pool(name="st", bufs=3, space="PSUM") as st_pool,
            tc.tile_pool(name="otp", bufs=2, space="PSUM") as ot_pool,
        ):
            identity = const_pool.tile([P, P], F32)
            make_identity(nc, identity)

            # prefetch every head's inputs up front: no-wait DMAs stream in
            # the background while compute proceeds
            knats, qnats, vtmps = [], [], []
            for h in range(HPC):
                knat = load_pool.tile([P, NT, D], F32, tag=f"knat{h}")
                nc.sync.dma_start(knat[:], k[h].rearrange("(t p) d -> p t d", p=P))
                # q loaded DOUBLED along a repeat dim (0-stride DRAM read):
                # transposing [128m, (2,64d)] then yields Q^T duplicated on
                # both partition halves, as the row-packed matmuls need
                qnat = load_pool.tile([P, NT, 2, D], F32, tag=f"qnat{h}")
                for r in range(2):
                    nc.sync.dma_start(
                        qnat[:, :, r, :],
                        q[h].rearrange("(t p) d -> p t d", p=P),
                    )
                vtmp = load_pool.tile([P, NT, D], F32, tag=f"vtmp{h}")
                nc.sync.dma_start(vtmp[:], v[h].rearrange("(t p) d -> p t d", p=P))
                knats.append(knat)
                qnats.append(qnat)
                vtmps.append(vtmp)

            heads = [{} for _ in range(HPC)]

            def setup_chunks(h):
                """Emission chunks for head h's setup, in dependency order."""
                st = heads[h]

                def scale_chain():
                    knat, vtmp = knats[h], vtmps[h]
                    ktmp = work_pool.tile([P, NT, D], F32, tag="ktmp")
                    nc.vector.tensor_mul(out=ktmp[:], in0=knat[:], in1=knat[:])
                    ksq = head_pool.tile([P, NT], F32, tag="ksq")
                    nc.vector.tensor_reduce(
                        ksq[:], ktmp[:],
                        axis=mybir.AxisListType.X, op=mybir.AluOpType.add,
                    )
                    w = head_pool.tile([P, NT], F32, tag="w")
                    nc.scalar.activation(
                        w[:], ksq[:], mybir.ActivationFunctionType.Exp, scale=-0.125
                    )
                    vaug = head_pool.tile([P, NT, D + 1], MM_DT, tag="vaug")
                    nc.vector.tensor_mul(
                        out=vaug[:, :, :D],
                        in0=vtmp[:],
                        in1=w[:, :, None].to_broadcast((P, NT, D)),
                    )
                    nc.vector.tensor_copy(out=vaug[:, :, D : D + 1], in_=w[:, :, None])
                    st["vaug"] = vaug
                    # kt: key-tile PAIRS stacked on partition halves
                    # (even tile at partitions 0:64, odd at 64:128) so two
                    # QK matmuls can row-pack the PE array concurrently.
                    st["kt"] = head_pool.tile(
                        [P, NT // 2, P], MM_DT, tag="kt", name="kt"
                    )
                    # qt: Q^T duplicated into both partition halves (the
                    # row-packed matmuls stream rhs partitions 0:64 and
                    # 64:128 into array row groups 0-1 and 2-3)
                    st["qt"] = head_pool.tile([P, NT, P], MM_DT, tag="qt", name="qt")

                def ktr_group(g):
                    # 4 pair-transposes: [128n, (2t, 64d)] -> [(2t, 64d), 128n]
                    # lands even tile at partitions 0:64, odd at 64:128
                    def run():
                        src = knats[h]
                        dst = heads[h]["kt"]
                        tp = st_pool.tile([P, 4, P], F32, tag="stg", name="tp")
                        for j in range(4):
                            pr = 4 * g + j
                            nc.tensor.transpose(
                                tp[:, j, :], src[:, 2 * pr : 2 * pr + 2, :],
                                identity[:],
                            )
                        nc.vector.tensor_copy(
                            out=dst[:, 4 * g : 4 * g + 4, :], in_=tp[:]
                        )

                    return run

                def qtr_group(g):
                    # transpose a 0-stride doubled view [128m, (2, 64d)] so
                    # the output holds Q^T duplicated on both partition
                    # halves (rows 0:64 and 64:128) in one shot
                    def run():
                        src = qnats[h]
                        dst = heads[h]["qt"]
                        tp = st_pool.tile([P, 4, P], F32, tag="stg", name="tp")
                        for j in range(4):
                            nc.tensor.transpose(
                                tp[:, j, :], src[:, 4 * g + j, :, :], identity[:]
                            )
                        nc.vector.tensor_copy(
                            out=dst[:, 4 * g : 4 * g + 4, :], in_=tp[:]
                        )

                    return run

                yield scale_chain
                # query block mb needs kt pair-groups up to (2mb+1)//4 and
                # qt group mb; yield in that dependency order
                yield ktr_group(0)
                yield qtr_group(0)
                yield qtr_group(1)
                yield ktr_group(1)
                yield qtr_group(2)
                yield qtr_group(3)

            def emit_mb(h, mb):
                kt, qt, vaug = heads[h]["kt"], heads[h]["qt"], heads[h]["vaug"]
                nsub = 4 * mb          # sub-diagonal key tiles
                qt_lo = qt[:D, 4 * mb : 4 * mb + 4, :]   # [64, 512]
                qt_hi = qt[D:, 4 * mb : 4 * mb + 4, :]   # [64, 512]
                ot = ot_pool.tile([D + 1, QB], F32, tag="ot")
                ntiles = nsub + 4

                # --- sub-diagonal tiles: unmasked, accumulate first ---
                prev = None
                for s in range(0, nsub, G):
                    stg = st_pool.tile([P, G, QB], F32, tag="stg")
                    pr = s // 2
                    nc.tensor.matmul(
                        stg[:, 0, :], kt[:D, pr, :], qt_lo,
                        start=True, stop=True, skip_group_check=True,
                    )
                    nc.tensor.matmul(
                        stg[:, 1, :], kt[D:, pr, :], qt_hi,
                        start=True, stop=True, skip_group_check=True,
                    )
                    pg = p_pool.tile([P, G, QB], MM_DT, tag="pg")
                    nc.scalar.activation(
                        pg[:], stg[:],
                        mybir.ActivationFunctionType.Exp, scale=0.25,
                    )
                    if prev is not None:
                        _emit_pv(nc, ot, vaug, prev, ntiles)
                    prev = (pg, [s, s + 1])
                if prev is not None:
                    _emit_pv(nc, ot, vaug, prev, ntiles)

                # --- diagonal block: 4 masked tiles, accumulated last ---
                pgd = p_pool.tile([P, 4, QB], MM_DT, tag="pgd")
                for a in range(2):
                    stg = st_pool.tile([P, G, QB], F32, tag="stg")
                    pr = 2 * mb + a
                    nc.tensor.matmul(
                        stg[:, 0, :], kt[:D, pr, :], qt_lo,
                        start=True, stop=True, skip_group_check=True,
                    )
                    nc.tensor.matmul(
                        stg[:, 1, :], kt[D:, pr, :], qt_hi,
                        start=True, stop=True, skip_group_check=True,
                    )
                    nc.scalar.activation(
                        pgd[:, 2 * a : 2 * a + 2, :], stg[:],
                        mybir.ActivationFunctionType.Exp, scale=0.25,
                    )
                    for j in range(G):
                        # keep pgd[n, jj, m] iff m - n - 128 jj >= 0
                        jj = 2 * a + j
                        nc.gpsimd.affine_select(
                            out=pgd[:, jj, :], in_=pgd[:, jj, :],
                            compare_op=mybir.AluOpType.is_ge, fill=0.0,
                            base=-P * jj, pattern=[[1, QB]],
                            channel_multiplier=-1,
                        )
                for j in range(4):
                    nc.tensor.matmul(
                        ot[:], vaug[:, 4 * mb + j, :], pgd[:, j, :],
                        start=(nsub == 0 and j == 0), stop=(j == 3),
                        skip_group_check=True,
                    )

                # ---------- epilogue: transpose + normalize ----------
                ot_sb = epi_pool.tile([D + 1, QB], F32, tag="ot_sb")
                nc.vector.tensor_copy(out=ot_sb[:], in_=ot[:])
                tpo = ot_pool.tile([P, 4, D + 1], F32, tag="ot", name="tpo")
                for j in range(4):
                    nc.tensor.transpose(
                        tpo[:, j, :],
                        ot_sb[:, j * P : (j + 1) * P],
                        identity[: D + 1, : D + 1],
                    )
                linv = epi_pool.tile([P, 4], F32, tag="linv")
                nc.vector.reciprocal(linv[:], tpo[:, :, D])
                o_sb = epi_pool.tile([P, 4, D], F32, tag="o_sb")
                for j in range(4):
                    nc.vector.tensor_scalar_mul(
                        o_sb[:, j, :], tpo[:, j, :D], linv[:, j : j + 1]
                    )
                nc.sync.dma_start(
                    out[h, mb * QB : (mb + 1) * QB, :].rearrange(
                        "(j p) d -> p j d", p=P
                    ),
                    o_sb[:],
                )

            # ---- software-pipelined emission ----
            pending = list(setup_chunks(0))
            for c in pending[:3]:  # scale chain, ktr0, qtr0
                c()
            pending = pending[3:]
            for h in range(HPC):
                if h + 1 < HPC:
                    pending += list(setup_chunks(h + 1))
                for mb in range(MBS):
                    emit_mb(h, mb)
                    if h + 1 == HPC and mb == MBS - 1:
                        take = len(pending)
                    else:
                        take = -(-len(pending) // (MBS - mb)) if pending else 0
                    for c in pending[:take]:
                        c()
                    pending = pending[take:]

    nc.compile()
    return nc


def _emit_pv(nc, ot, vaug, group, ntiles):
    pg, tiles = group
    for j, nt in enumerate(tiles):
        nc.tensor.matmul(
            ot[:],
            vaug[:, nt, :],
            pg[:, j, :],
            start=(nt == 0),
            stop=(nt == ntiles - 1),
            skip_group_check=True,
        )


_NC = None


def _get_nc():
    global _NC
    if _NC is None:
        _NC = build_nc()
    return _NC


def kernel(q: np.ndarray, k: np.ndarray, v: np.ndarray) -> np.ndarray:
    from concourse.bass_utils import run_bass_kernel_spmd

    nc = _get_nc()
    qf = np.ascontiguousarray(np.asarray(q, dtype=np.float32).reshape(B * H, N, D))
    kf = np.ascontiguousarray(np.asarray(k, dtype=np.float32).reshape(B * H, N, D))
    vf = np.ascontiguousarray(np.asarray(v, dtype=np.float32).reshape(B * H, N, D))
    in_maps = [
        {
            "q": np.ascontiguousarray(qf[c * HPC : (c + 1) * HPC]),
            "k": np.ascontiguousarray(kf[c * HPC : (c + 1) * HPC]),
            "v": np.ascontiguousarray(vf[c * HPC : (c + 1) * HPC]),
        }
        for c in range(NCORES)
    ]
    res = run_bass_kernel_spmd(nc, in_maps, core_ids=list(range(NCORES)))
    outs = [res.results[c]["out"] for c in range(NCORES)]
    return np.concatenate(outs, axis=0).reshape(B, H, N, D)


if __name__ == "__main__":
    rng = np.random.default_rng(0)
    qq = rng.standard_normal((B, H, N, D), dtype=np.float32)
    kk = rng.standard_normal((B, H, N, D), dtype=np.float32)
    vv = rng.standard_normal((B, H, N, D), dtype=np.float32)
    o = kernel(q=qq, k=kk, v=vv)
    print("kernel ran, out shape", o.shape, "finite:", np.isfinite(o).all())


# revision 22
# speedup vs baseline: 1.0515x; 1.0515x over previous
"""RBF-kernel causal attention on 8 Trainium2 NeuronCores.

B=2, H=16, N=2048, D=64. Shards the 32 (b,h) attention instances across 8
cores (4 heads per core). Math notes:

  logits = -relu(||q-k||^2)/sqrt(D); relu is a no-op (||q-k||^2 >= 0 up to
  rounding), and softmax is invariant to per-query offsets, so
      softmax_n(-(qsq_m + ksq_n - 2 qk)/8) == softmax_n(qk/4 - ksq_n/8)
  We compute P'' = exp(0.25 * K Q^T) in a [key, query] layout and fold the
  exp(-0.125 ksq_n) per-key factor into V (and into the appended ones-column
  that produces the softmax denominator):
      [O^T | l] accumulates via matmul(lhsT=V_aug_scaled, rhs=P'').
  Final output O[m,d] = OT[d,m] / l[m], un-transposed via PE transpose.

Emission is manually software-pipelined: head h+1's setup chunks (transposes,
ksq, V scaling) are interleaved between head h's query blocks so the tile
scheduler (limited lookahead) can overlap them.
"""

import sys

if "/opt/trn_rl_repo" not in sys.path:
    sys.path.insert(0, "/opt/trn_rl_repo")

import numpy as np

import concourse.bacc as bacc
import concourse.mybir as mybir
import concourse.tile as tile
from concourse.masks import make_identity

B, H, N, D = 2, 16, 2048, 64
NCORES = 8
HPC = (B * H) // NCORES  # heads per core = 4
P = 128                  # partitions
NT = N // P              # key tiles per head = 16
QB = 512                 # query block (matmul moving dim)
MBS = N // QB            # query blocks per head = 4
G = 2                    # key tiles per exp/ACT group (2 PSUM banks)

F32 = mybir.dt.float32
# float32r = relaxed-precision fp32 matmul (1 cycle/row at moving dim >= 256
# instead of 4 for float32)
MM_DT = mybir.dt.float32r


def build_nc():
    nc = bacc.Bacc("TRN2", target_bir_lowering=False, debug=False)
    q = nc.dram_tensor("q", [HPC, N, D], F32, kind="ExternalInput")
    k = nc.dram_tensor("k", [HPC, N, D], F32, kind="ExternalInput")
    v = nc.dram_tensor("v", [HPC, N, D], F32, kind="ExternalInput")
    out = nc.dram_tensor("out", [HPC, N, D], F32, kind="ExternalOutput")

    with tile.TileContext(nc) as tc:
        with (
            tc.tile_pool(name="const", bufs=1) as const_pool,
            tc.tile_pool(name="loads", bufs=1) as load_pool,
            tc.tile_pool(name="head", bufs=2) as head_pool,
            tc.tile_pool(name="work", bufs=3) as work_pool,
            tc.tile_pool(name="p", bufs=4) as p_pool,
            tc.tile_pool(name="epi", bufs=3) as epi_pool,
            tc.tile_pool(name="st", bufs=3, space="PSUM") as st_pool,
            tc.tile_pool(name="otp", bufs=2, space="PSUM") as ot_pool,
        ):
            identity = const_pool.tile([P, P], F32)
            make_identity(nc, identity)

            # prefetch every head's inputs up front: no-wait DMAs stream in
            # the background while compute proceeds
            knats, qnats, vtmps = [], [], []
            for h in range(HPC):
                knat = load_pool.tile([P, NT, D], F32, tag=f"knat{h}")
                nc.sync.dma_start(knat[:], k[h].rearrange("(t p) d -> p t d", p=P))
                # q loaded DOUBLED along a repeat dim (0-stride DRAM read):
                # transposing [128m, (2,64d)] then yields Q^T duplicated on
                # both partition halves, as the row-packed matmuls need
                qnat = load_pool.tile([P, NT, 2, D], F32, tag=f"qnat{h}")
                for r in range(2):
                    nc.sync.dma_start(
                        qnat[:, :, r, :],
                        q[h].rearrange("(t p) d -> p t d", p=P),
                    )
                vtmp = load_pool.tile([P, NT, D], F32, tag=f"vtmp{h}")
                nc.sync.dma_start(vtmp[:], v[h].rearrange("(t p) d -> p t d", p=P))
                knats.append(knat)
                qnats.append(qnat)
                vtmps.append(vtmp)

            heads = [{} for _ in range(HPC)]

            def setup_chunks(h):
                """Emission chunks for head h's setup, in dependency order."""
                st = heads[h]

                def scale_chain():
                    knat, vtmp = knats[h], vtmps[h]
                    ktmp = work_pool.tile([P, NT, D], F32, tag="ktmp")
                    nc.vector.tensor_mul(out=ktmp[:], in0=knat[:], in1=knat[:])
                    ksq = head_pool.tile([P, NT], F32, tag="ksq")
                    nc.vector.tensor_reduce(
                        ksq[:], ktmp[:],
                        axis=mybir.AxisListType.X, op=mybir.AluOpType.add,
                    )
                    w = head_pool.tile([P, NT], F32, tag="w")
                    nc.scalar.activation(
                        w[:], ksq[:], mybir.ActivationFunctionType.Exp, scale=-0.125
                    )
                    vaug = head_pool.tile([P, NT, D + 1], MM_DT, tag="vaug")
                    nc.vector.tensor_mul(
                        out=vaug[:, :, :D],
                        in0=vtmp[:],
                        in1=w[:, :, None].to_broadcast((P, NT, D)),
                    )
                    nc.vector.tensor_copy(out=vaug[:, :, D : D + 1], in_=w[:, :, None])
                    st["vaug"] = vaug
                    # kt: key-tile PAIRS stacked on partition halves
                    # (even tile at partitions 0:64, odd at 64:128) so two
                    # QK matmuls can row-pack the PE array concurrently.
                    st["kt"] = head_pool.tile(
                        [P, NT // 2, P], MM_DT, tag="kt", name="kt"
                    )
                    # qt: Q^T duplicated into both partition halves (the
                    # row-packed matmuls stream rhs partitions 0:64 and
                    # 64:128 into array row groups 0-1 and 2-3)
                    st["qt"] = head_pool.tile([P, NT, P], MM_DT, tag="qt", name="qt")

                def ktr_group(g):
                    # 4 pair-transposes: [128n, (2t, 64d)] -> [(2t, 64d), 128n]
                    # lands even tile at partitions 0:64, odd at 64:128
                    def run():
                        src = knats[h]
                        dst = heads[h]["kt"]
                        tp = st_pool.tile([P, 4, P], F32, tag="stg", name="tp")
                        for j in range(4):
                            pr = 4 * g + j
                            nc.tensor.transpose(
                                tp[:, j, :], src[:, 2 * pr : 2 * pr + 2, :],
                                identity[:],
                            )
                        nc.vector.tensor_copy(
                            out=dst[:, 4 * g : 4 * g + 4, :], in_=tp[:]
                        )

                    return run

                def qtr_group(g):
                    # transpose a 0-stride doubled view [128m, (2, 64d)] so
                    # the output holds Q^T duplicated on both partition
                    # halves (rows 0:64 and 64:128) in one shot
                    def run():
                        src = qnats[h]
                        dst = heads[h]["qt"]
                        tp = st_pool.tile([P, 4, P], F32, tag="stg", name="tp")
                        for j in range(4):
                            nc.tensor.transpose(
                                tp[:, j, :], src[:, 4 * g + j, :, :], identity[:]
                            )
                        nc.vector.tensor_copy(
                            out=dst[:, 4 * g : 4 * g + 4, :], in_=tp[:]
                        )

                    return run

                yield scale_chain
                # query block mb needs kt pair-groups up to (2mb+1)//4 and
                # qt group mb; yield in that dependency order
                yield ktr_group(0)
                yield qtr_group(0)
                yield qtr_group(1)
                yield ktr_group(1)
                yield qtr_group(2)
                yield qtr_group(3)

            def emit_mb(h, mb):
                kt, qt, vaug = heads[h]["kt"], heads[h]["qt"], heads[h]["vaug"]
                nsub = 4 * mb          # sub-diagonal key tiles
                qt_lo = qt[:D, 4 * mb : 4 * mb + 4, :]   # [64, 512]
                qt_hi = qt[D:, 4 * mb : 4 * mb + 4, :]   # [64, 512]
                ot = ot_pool.tile([D + 1, QB], F32, tag="ot")
                ntiles = nsub + 4

                # --- sub-diagonal tiles: unmasked, accumulate first ---
                prev = None
                for s in range(0, nsub, G):
                    stg = st_pool.tile([P, G, QB], F32, tag="stg")
                    pr = s // 2
                    nc.tensor.matmul(
                        stg[:, 0, :], kt[:D, pr, :], qt_lo,
                        start=True, stop=True, skip_group_check=True,
                    )
                    nc.tensor.matmul(
                        stg[:, 1, :], kt[D:, pr, :], qt_hi,
                        start=True, stop=True, skip_group_check=True,
                    )
                    pg = p_pool.tile([P, G, QB], MM_DT, tag="pg")
                    nc.scalar.activation(
                        pg[:], stg[:],
                        mybir.ActivationFunctionType.Exp, scale=0.25,
                    )
                    if prev is not None:
                        _emit_pv(nc, ot, vaug, prev, ntiles)
                    prev = (pg, [s, s + 1])
                if prev is not None:
                    _emit_pv(nc, ot, vaug, prev, ntiles)

                # --- diagonal block: 4 masked tiles, accumulated last ---
                pgd = p_pool.tile([P, 4, QB], MM_DT, tag="pgd")
                for a in range(2):
                    # columns m < 128*(2a) of tiles (2a, 2a+1) are fully
                    # masked: skip their QK matmul + exp; affine_select
                    # below zero-fills that (otherwise garbage) region.
                    c0 = P * 2 * a
                    stg = st_pool.tile([P, G, QB], F32, tag="stg")
                    pr = 2 * mb + a
                    nc.tensor.matmul(
                        stg[:, 0, c0:],
                        kt[:D, pr, :],
                        qt[:D, 4 * mb + 2 * a : 4 * mb + 4, :],
                        start=True, stop=True, skip_group_check=True,
                    )
                    nc.tensor.matmul(
                        stg[:, 1, c0:],
                        kt[D:, pr, :],
                        qt[D:, 4 * mb + 2 * a : 4 * mb + 4, :],
                        start=True, stop=True, skip_group_check=True,
                    )
                    nc.scalar.activation(
                        pgd[:, 2 * a : 2 * a + 2, c0:], stg[:, :, c0:],
                        mybir.ActivationFunctionType.Exp, scale=0.25,
                    )
                    for j in range(G):
                        # keep pgd[n, jj, m] iff m - n - 128 jj >= 0
                        jj = 2 * a + j
                        nc.gpsimd.affine_select(
                            out=pgd[:, jj, :], in_=pgd[:, jj, :],
                            compare_op=mybir.AluOpType.is_ge, fill=0.0,
                            base=-P * jj, pattern=[[1, QB]],
                            channel_multiplier=-1,
                        )
                for j in range(4):
                    nc.tensor.matmul(
                        ot[:], vaug[:, 4 * mb + j, :], pgd[:, j, :],
                        start=(nsub == 0 and j == 0), stop=(j == 3),
                        skip_group_check=True,
                    )

                # ---------- epilogue: transpose + normalize ----------
                ot_sb = epi_pool.tile([D + 1, QB], F32, tag="ot_sb")
                nc.vector.tensor_copy(out=ot_sb[:], in_=ot[:])
                tpo = ot_pool.tile([P, 4, D + 1], F32, tag="ot", name="tpo")
                for j in range(4):
                    nc.tensor.transpose(
                        tpo[:, j, :],
                        ot_sb[:, j * P : (j + 1) * P],
                        identity[: D + 1, : D + 1],
                    )
                linv = epi_pool.tile([P, 4], F32, tag="linv")
                nc.vector.reciprocal(linv[:], tpo[:, :, D])
                o_sb = epi_pool.tile([P, 4, D], F32, tag="o_sb")
                for j in range(4):
                    nc.vector.tensor_scalar_mul(
                        o_sb[:, j, :], tpo[:, j, :D], linv[:, j : j + 1]
                    )
                nc.sync.dma_start(
                    out[h, mb * QB : (mb + 1) * QB, :].rearrange(
                        "(j p) d -> p j d", p=P
                    ),
                    o_sb[:],
                )

            # ---- software-pipelined emission ----
            pending = list(setup_chunks(0))
            for c in pending[:3]:  # scale chain, ktr0, qtr0
                c()
            pending = pending[3:]
            for h in range(HPC):
                if h + 1 < HPC:
                    pending += list(setup_chunks(h + 1))
                for mb in range(MBS):
                    emit_mb(h, mb)
                    if h + 1 == HPC and mb == MBS - 1:
                        take = len(pending)
                    else:
                        take = -(-len(pending) // (MBS - mb)) if pending else 0
                    for c in pending[:take]:
                        c()
                    pending = pending[take:]

    nc.compile()
    return nc


def _emit_pv(nc, ot, vaug, group, ntiles):
    pg, tiles = group
    for j, nt in enumerate(tiles):
        nc.tensor.matmul(
            ot[:],
            vaug[:, nt, :],
            pg[:, j, :],
            start=(nt == 0),
            stop=(nt == ntiles - 1),
            skip_group_check=True,
        )


_NC = None


def _get_nc():
    global _NC
    if _NC is None:
        _NC = build_nc()
    return _NC


def kernel(q: np.ndarray, k: np.ndarray, v: np.ndarray) -> np.ndarray:
    from concourse.bass_utils import run_bass_kernel_spmd

    nc = _get_nc()
    qf = np.ascontiguousarray(np.asarray(q, dtype=np.float32).reshape(B * H, N, D))
    kf = np.ascontiguousarray(np.asarray(k, dtype=np.float32).reshape(B * H, N, D))
    vf = np.ascontiguousarray(np.asarray(v, dtype=np.float32).reshape(B * H, N, D))
    in_maps = [
        {
            "q": np.ascontiguousarray(qf[c * HPC : (c + 1) * HPC]),
            "k": np.ascontiguousarray(kf[c * HPC : (c + 1) * HPC]),
            "v": np.ascontiguousarray(vf[c * HPC : (c + 1) * HPC]),
        }
        for c in range(NCORES)
    ]
    res = run_bass_kernel_spmd(nc, in_maps, core_ids=list(range(NCORES)))
    outs = [res.results[c]["out"] for c in range(NCORES)]
    return np.concatenate(outs, axis=0).reshape(B, H, N, D)


if __name__ == "__main__":
    rng = np.random.default_rng(0)
    qq = rng.standard_normal((B, H, N, D), dtype=np.float32)
    kk = rng.standard_normal((B, H, N, D), dtype=np.float32)
    vv = rng.standard_normal((B, H, N, D), dtype=np.float32)
    o = kernel(q=qq, k=kk, v=vv)
    print("kernel ran, out shape", o.shape, "finite:", np.isfinite(o).all())
